# revision 13
# baseline (speedup 1.0000x reference)
"""Trainium2 Bass kernel for CompositionModel (gnn_message_passing).

Model: per-cell MLP over [log1p(X) ++ Z[cell_to_batch]] followed by a
segment-mean over batch labels.

Strategy:
  * Host: sort cells by segment id, pad each segment run to a multiple of 64
    so every 64-cell "minichunk" is single-segment; gather Z rows per cell;
    ship everything transposed (features on partitions) in bf16, blocked as
    [P, 512]-column blocks; two blocks share one DMA/log1p pass.
  * Device (8 cores, data-parallel over cells, identical static program):
      log1p (ACT Ln, 1024 cols/op) -> L1 matmul (K=128 X-part + K=32 Z-part,
      bf16) -> bias+ReLU -> fp8 h1 -> L2 as fp8 DoubleRow matmuls against
      W2 split into a (hi, lo) fp8 pair sharing one x64 scale (W2 is then
      effectively exact; only h1 carries fp8 rounding, which averages out
      in the segment mean) -> fused bias+ReLU+cast on DVE -> GpSimd
      pre-folds each minichunk in half -> grouped DVE tensor_reduce.
      The third (linear) layer commutes with the segment sum and is applied
    on the host to the 512x256 segment sums instead of 500k cells.
  * Host epilogue: subtract the (identical, analytically known) contribution
    of pad cells, scatter-add minichunk sums into segment sums, undo the x64
    W2 scale, apply W3/b3 and divide by true counts.
"""

import numpy as np
import ml_dtypes

import concourse.bacc as bacc
import concourse.mybir as mybir
import concourse.tile as tile
from concourse.bass_utils import run_bass_kernel_spmd

BF16 = ml_dtypes.bfloat16
FP8 = ml_dtypes.float8_e4m3fn

N_CORES = 8
DX = 128
DZ = 32
H = 256
B = 512
MC = 64            # minichunk: cells per single-segment group
BLK = 512          # cells per device block (matmul moving free dim)
NBLK = 126         # blocks per core (fits the fixed reference input)
W2SCALE = 64.0     # fp8 pre-scale on W2/b2, divided out on the host

_compiled = {}
_last_in_maps = None


def _build_program(nblk):
    f32 = mybir.dt.float32
    bf16 = mybir.dt.bfloat16
    fp8 = mybir.dt.float8e4
    Alu = mybir.AluOpType
    Act = mybir.ActivationFunctionType
    DR = mybir.MatmulPerfMode.DoubleRow
    mc_per_core = nblk * (BLK // MC)

    nc = bacc.Bacc("TRN2", target_bir_lowering=False, debug=False,
                   num_devices=N_CORES)

    xt_d = nc.dram_tensor("xt", [nblk // 2, DX, 2 * BLK], bf16,
                          kind="ExternalInput")
    zct_d = nc.dram_tensor("zct", [nblk, DZ, BLK], bf16, kind="ExternalInput")
    w1x_d = nc.dram_tensor("w1x", [DX, H], bf16, kind="ExternalInput")
    w1z_d = nc.dram_tensor("w1z", [DZ, H], bf16, kind="ExternalInput")
    # [m-half][hi/lo][p, ktile*128] fp8, pre-scaled by W2SCALE
    w2_d = nc.dram_tensor("w2", [2, 2, 128, 2 * 128], fp8,
                          kind="ExternalInput")
    b1_d = nc.dram_tensor("b1", [2, 128, 1], f32, kind="ExternalInput")
    b2_d = nc.dram_tensor("b2", [2, 128, 1], f32, kind="ExternalInput")
    out_d = nc.dram_tensor("out", [128, 2 * mc_per_core], f32,
                           kind="ExternalOutput")

    with tile.TileContext(nc) as tc:
        with tc.tile_pool(name="consts", bufs=1) as cpool, \
             tc.tile_pool(name="work", bufs=4) as pool, \
             tc.tile_pool(name="psum", bufs=2, space="PSUM") as psum:

            w1xa = cpool.tile([DX, 128], bf16, tag="w1xa")
            w1xb = cpool.tile([DX, 128], bf16, tag="w1xb")
            nc.sync.dma_start(w1xa[:], w1x_d[:, 0:128])
            nc.sync.dma_start(w1xb[:], w1x_d[:, 128:256])
            w1za = cpool.tile([DZ, 128], bf16, tag="w1za")
            w1zb = cpool.tile([DZ, 128], bf16, tag="w1zb")
            nc.sync.dma_start(w1za[:], w1z_d[:, 0:128])
            nc.sync.dma_start(w1zb[:], w1z_d[:, 128:256])
            w2t = {}
            for m in range(2):
                for t in range(2):
                    w = cpool.tile([128, 2 * 128], fp8, tag=f"w2_{m}{t}")
                    nc.sync.dma_start(w[:], w2_d[m, t])
                    w2t[m, t] = w[:].rearrange("p (k m) -> p k m", k=2)
            b1a = cpool.tile([128, 1], f32, tag="b1a")
            b1b = cpool.tile([128, 1], f32, tag="b1b")
            b2a = cpool.tile([128, 1], f32, tag="b2a")
            b2b = cpool.tile([128, 1], f32, tag="b2b")
            nc.sync.dma_start(b1a[:], b1_d[0])
            nc.sync.dma_start(b1b[:], b1_d[1])
            nc.sync.dma_start(b2a[:], b2_d[0])
            nc.sync.dma_start(b2b[:], b2_d[1])
            ones = cpool.tile([128, 1], f32, tag="ones")
            nc.vector.memset(ones[:], 1.0)

            out2 = cpool.tile([128, 2 * mc_per_core], f32, tag="out2")

            # two blocks share one DMA + one Ln op (amortize ACT overhead);
            # the Ln for superblock k+1 is emitted mid-way through k so the
            # ACT engine computes it before it gates the next L1 matmuls
            def emit_ln(k):
                xt = pool.tile([DX, 2 * BLK], bf16, tag="xt")
                nc.sync.dma_start(xt[:], xt_d[k])
                xl = pool.tile([DX, 2 * BLK], bf16, tag="xl")
                nc.scalar.activation(xl[:], xt[:], Act.Ln, bias=ones[:])
                return xl

            nsb = nblk // 2
            xl_cur = emit_ln(0)
            for sblk in range(nsb):
                xl_next = None
                for half in range(2):
                    if half == 1 and sblk + 1 < nsb:
                        xl_next = emit_ln(sblk + 1)
                    blk = 2 * sblk + half
                    xls = xl_cur[:, half * BLK:(half + 1) * BLK]
                    zct = pool.tile([DZ, BLK], bf16, tag="zct")
                    nc.sync.dma_start(zct[:], zct_d[blk])

                    ps1a = psum.tile([128, BLK], f32, tag="ps1a")
                    nc.tensor.matmul(ps1a[:], w1xa[:], xls, start=True, stop=False)
                    nc.tensor.matmul(ps1a[:], w1za[:], zct[:], start=False, stop=True)
                    ps1b = psum.tile([128, BLK], f32, tag="ps1b")
                    nc.tensor.matmul(ps1b[:], w1xb[:], xls, start=True, stop=False)
                    nc.tensor.matmul(ps1b[:], w1zb[:], zct[:], start=False, stop=True)

                    # h1 halves stacked as the two DoubleRow k-tiles, fp8
                    h1 = pool.tile([128, 2 * BLK], fp8, tag="h1")
                    nc.scalar.activation(h1[:, 0:BLK], ps1a[:], Act.Relu,
                                         bias=b1a[:])
                    nc.scalar.activation(h1[:, BLK:2 * BLK], ps1b[:], Act.Relu,
                                         bias=b1b[:])
                    h1v = h1[:].rearrange("p (k c) -> p k c", k=2)

                    ps2a = psum.tile([128, BLK], f32, tag="ps2a")
                    nc.tensor.matmul(ps2a[:], w2t[0, 0], h1v, start=True,
                                     stop=False, perf_mode=DR)
                    nc.tensor.matmul(ps2a[:], w2t[0, 1], h1v, start=False,
                                     stop=True, perf_mode=DR)
                    ps2b = psum.tile([128, BLK], f32, tag="ps2b")
                    nc.tensor.matmul(ps2b[:], w2t[1, 0], h1v, start=True,
                                     stop=False, perf_mode=DR)
                    nc.tensor.matmul(ps2b[:], w2t[1, 1], h1v, start=False,
                                     stop=True, perf_mode=DR)

                    h2 = pool.tile([128, 2 * BLK], bf16, tag="h2")
                    nc.vector.tensor_scalar(h2[:, 0:BLK], ps2a[:], b2a[:], 0.0,
                                            op0=Alu.add, op1=Alu.max)
                    nc.vector.tensor_scalar(h2[:, BLK:2 * BLK], ps2b[:], b2b[:],
                                            0.0, op0=Alu.add, op1=Alu.max)

                    # GpSimd pre-folds each 64-cell minichunk in half
                    # (SBUF->SBUF add), halving the DVE reduce read size.
                    h2v = h2[:].rearrange("p (g t m) -> p g t m", t=2, m=MC // 2)
                    h2f = pool.tile([128, BLK], bf16, tag="h2f")
                    h2fv = h2f[:].rearrange("p (g m) -> p g m", m=MC // 2)
                    nc.gpsimd.tensor_tensor(
                        h2fv, h2v[:, :, 0:1, :], h2v[:, :, 1:2, :], op=Alu.add)

                    oslice = slice(blk * 2 * (BLK // MC),
                                   (blk + 1) * 2 * (BLK // MC))
                    nc.vector.tensor_reduce(
                        out2[:, oslice], h2fv,
                        axis=mybir.AxisListType.X, op=Alu.add)
                xl_cur = xl_next

            nc.sync.dma_start(out_d[:], out2[:])

    nc.compile()
    return nc


def _get_program(nblk):
    if nblk not in _compiled:
        _compiled[nblk] = _build_program(nblk)
    return _compiled[nblk]


def kernel(X, Z, W1, b1, W2, b2, W3, b3, cell_to_batch, sample_idx_batch):
    X = np.asarray(X)
    Z = np.asarray(Z)
    W1 = np.asarray(W1, dtype=np.float32)
    b1 = np.asarray(b1, dtype=np.float32)
    W2 = np.asarray(W2, dtype=np.float32)
    b2 = np.asarray(b2, dtype=np.float32)
    W3 = np.asarray(W3, dtype=np.float32)
    b3 = np.asarray(b3, dtype=np.float32)
    c2b = np.asarray(cell_to_batch).astype(np.int64)
    sib = np.asarray(sample_idx_batch).astype(np.int64)

    n = X.shape[0]
    nseg = sib.shape[0]
    seg = sib[c2b]

    # ---- host layout prep -------------------------------------------------
    order = np.argsort(seg, kind="stable")
    seg_sorted = seg[order]
    counts = np.bincount(seg, minlength=nseg).astype(np.int64)
    padded = ((counts + MC - 1) // MC) * MC
    starts = np.concatenate([[0], np.cumsum(padded)])[:nseg]
    total_pad = int(padded.sum())
    nblk = NBLK
    while total_pad > N_CORES * nblk * BLK:  # safety fallback, recompiles
        nblk += 2
    ntot = N_CORES * nblk * BLK
    mc_per_core = nblk * (BLK // MC)
    run_starts = np.concatenate([[0], np.cumsum(counts)])[:nseg]
    ranks = np.arange(n, dtype=np.int64) - run_starts[seg_sorted]
    slots = starts[seg_sorted] + ranks

    Xs = np.zeros((ntot, DX), dtype=BF16)
    Xs[slots] = X[order].astype(BF16)
    Zs = np.zeros((ntot, DZ), dtype=BF16)
    Zs[slots] = Z[c2b[order]].astype(BF16)

    xt = np.ascontiguousarray(
        Xs.reshape(N_CORES, nblk // 2, 2 * BLK, DX).transpose(0, 1, 3, 2))
    zct = np.ascontiguousarray(
        Zs.reshape(N_CORES, nblk, BLK, DZ).transpose(0, 1, 3, 2))

    n_mc = ntot // MC
    mc_label = np.full(n_mc, -1, dtype=np.int64)
    mc_real = np.zeros(n_mc, dtype=np.int64)
    mc_of_slot = slots // MC
    mc_label[mc_of_slot] = seg_sorted
    np.add.at(mc_real, mc_of_slot, 1)

    # ---- weights ----------------------------------------------------------
    w1x = np.ascontiguousarray(W1[:DX]).astype(BF16)
    w1z = np.ascontiguousarray(W1[DX:DX + DZ]).astype(BF16)
    # W2 as a scaled fp8 (hi, lo) pair; together they are W2 to ~4e-4
    w2f = W2.astype(BF16).astype(np.float32) * W2SCALE
    t_hi = w2f.astype(FP8)
    t_lo = (w2f - t_hi.astype(np.float32)).astype(FP8)
    w2q = np.zeros((2, 2, 128, 2 * 128), dtype=FP8)
    for m in range(2):
        for t, term in enumerate((t_hi, t_lo)):
            # [p, ktile*128] with element [p, k*128+mc] = term[k*128+p, m*128+mc]
            w2q[m, t] = (term.reshape(2, 128, H).transpose(1, 0, 2)
                         [:, :, m * 128:(m + 1) * 128].reshape(128, 256))
    b1d = np.ascontiguousarray(b1.reshape(2, 128, 1))
    b2d = np.ascontiguousarray(b2.reshape(2, 128, 1)) * W2SCALE

    # ---- run on 8 cores ---------------------------------------------------
    nc = _get_program(nblk)
    in_maps = []
    for c in range(N_CORES):
        in_maps.append({
            "xt": xt[c], "zct": zct[c],
            "w1x": w1x, "w1z": w1z, "w2": w2q, "b1": b1d, "b2": b2d,
        })
    global _last_in_maps
    _last_in_maps = in_maps
    res = run_bass_kernel_spmd(nc, in_maps, list(range(N_CORES)))

    # ---- host epilogue ----------------------------------------------------
    per_core = []
    for c in range(N_CORES):
        o = res.results[c]["out"].reshape(128, nblk, 2, BLK // MC)
        per_core.append(np.concatenate(
            [o[:, :, 0, :].reshape(128, mc_per_core),
             o[:, :, 1, :].reshape(128, mc_per_core)], axis=0))
    sums = np.concatenate(per_core, axis=1)  # [256, n_mc], scaled by W2SCALE

    # analytic contribution of one pad cell (X=0, Z=0), matching device math
    h1p = np.maximum(b1, 0.0).astype(FP8).astype(np.float32)
    w2eff = t_hi.astype(np.float32) + t_lo.astype(np.float32)
    h2p = np.maximum(h1p @ w2eff + W2SCALE * b2, 0.0)
    v_pad = h2p.astype(BF16).astype(np.float32)  # [256]
    sums = sums - np.outer(v_pad, (MC - mc_real).astype(np.float32))
    sums /= W2SCALE

    valid = mc_label >= 0
    S = np.zeros((nseg, H), dtype=np.float32)
    np.add.at(S, mc_label[valid], sums[:, valid].T)

    denom = np.maximum(counts, 1).astype(np.float32)[:, None]
    Y = S @ W3 / denom + b3[None, :]
    Y[counts == 0] = 0.0
    return Y.astype(np.float32)
